# revision 1
# baseline (speedup 1.0000x reference)
"""Sliding-window KV-cache update (concat along seq, keep last MAX_LEN) on 8 trn2 cores.

Full-input contract: kernel(**inputs) takes the unsharded (2, 32, 8192, 128)
bf16 caches plus (2, 32, 16, 128) new k/v, and returns the full
(new_k, new_v) pair.  Internally the work is sharded across 8 NeuronCores
along the num_heads axis (32 heads -> 4 per core); each (batch, head) slab is
fully independent, so per core the kernel is just two big shifted DRAM->DRAM
DMA copies (bulk: out[:, :8176, :] = cache[:, 16:, :]) plus two tiny tail
copies from k_new / v_new.
"""

import numpy as np

N_CORES = 8
B, H, S, D = 2, 32, 8192, 128
S_NEW = 16
KEEP = S - S_NEW  # 8176
HPC = H // N_CORES  # heads per core
BLK = B * HPC  # independent (batch, head) slabs per core

_NC_CACHE = {}


def _build_nc():
    """Build the single-core Bass program (same program on all 8 cores)."""
    import concourse.bass as bass
    import concourse.mybir as mybir

    nc = bass.Bass()
    dt = mybir.dt.bfloat16
    ck = nc.dram_tensor("cache_k", [BLK, S, D], dt, kind="ExternalInput")
    cv = nc.dram_tensor("cache_v", [BLK, S, D], dt, kind="ExternalInput")
    kn = nc.dram_tensor("k_new", [BLK, S_NEW, D], dt, kind="ExternalInput")
    vn = nc.dram_tensor("v_new", [BLK, S_NEW, D], dt, kind="ExternalInput")
    ok = nc.dram_tensor("out_k", [BLK, S, D], dt, kind="ExternalOutput")
    ov = nc.dram_tensor("out_v", [BLK, S, D], dt, kind="ExternalOutput")

    # Both HWDGE rings (sync=SP, scalar=ACT) spray over the same 8 SDMA
    # engines of this core's bank (measured: the second half of the bank is
    # unreachable for bulk data, and SWDGE descriptor generation caps at
    # ~65 GB/s).  The k/v split across the two rings just parallelizes
    # descriptor generation; the engines round-robin both rings at packet
    # granularity and stay ~98% busy at their ~27.2 GB/s line rate.
    with nc.Block() as block, nc.semaphore("dma_sem") as dma_sem:

        @block.sync
        def _(sync):
            # Bulk shifted copy: one DMA, 8 slabs x 8176 rows x 128
            # (contiguous 2 MiB runs, split by bass into 64 KiB descriptors).
            sync.dma_start(out=ok[:, 0:KEEP, :], in_=ck[:, S_NEW:S, :]).then_inc(
                dma_sem, 16
            )
            # Tail: the 16 new rows per slab (32 KiB total).
            sync.dma_start(out=ok[:, KEEP:S, :], in_=kn[:, :, :]).then_inc(dma_sem, 16)
            sync.wait_ge(dma_sem, 64)

        @block.scalar
        def _(scalar):
            scalar.dma_start(out=ov[:, 0:KEEP, :], in_=cv[:, S_NEW:S, :]).then_inc(
                dma_sem, 16
            )
            scalar.dma_start(out=ov[:, KEEP:S, :], in_=vn[:, :, :]).then_inc(
                dma_sem, 16
            )

    return nc


def _get_nc():
    if "nc" not in _NC_CACHE:
        _NC_CACHE["nc"] = _build_nc()
    return _NC_CACHE["nc"]


def _shard(arr, c, n_rows):
    """Head-shard for core c, flattened to (BLK, n_rows, D), contiguous."""
    sl = arr[:, c * HPC : (c + 1) * HPC]
    return np.ascontiguousarray(sl).reshape(BLK, n_rows, D)


def _run_spmd(cache_k, cache_v, k_new, v_new, trace=False, trace_kwargs=None):
    from concourse.bass_utils import run_bass_kernel_spmd

    nc = _get_nc()
    in_maps = [
        {
            "cache_k": _shard(cache_k, c, S),
            "cache_v": _shard(cache_v, c, S),
            "k_new": _shard(k_new, c, S_NEW),
            "v_new": _shard(v_new, c, S_NEW),
        }
        for c in range(N_CORES)
    ]
    kw = {}
    if trace:
        kw["trace"] = True
        if trace_kwargs:
            kw.update(trace_kwargs)
    return run_bass_kernel_spmd(nc, in_maps, core_ids=list(range(N_CORES)), **kw)


def _gather(results):
    out_k = np.concatenate(
        [results[c]["out_k"].reshape(B, HPC, S, D) for c in range(N_CORES)], axis=1
    )
    out_v = np.concatenate(
        [results[c]["out_v"].reshape(B, HPC, S, D) for c in range(N_CORES)], axis=1
    )
    return out_k, out_v


def kernel(cache_k, cache_v, k_new, v_new):
    cache_k = np.asarray(cache_k)
    cache_v = np.asarray(cache_v)
    k_new = np.asarray(k_new)
    v_new = np.asarray(v_new)
    res = _run_spmd(cache_k, cache_v, k_new, v_new)
    return _gather(res.results)



# revision 3
# speedup vs baseline: 1.7348x; 1.7348x over previous
"""Sliding-window KV-cache update (concat along seq, keep last MAX_LEN) on 8 trn2 cores.

Full-input contract: kernel(**inputs) takes the unsharded (2, 32, 8192, 128)
bf16 caches plus (2, 32, 16, 128) new k/v, and returns the full
(new_k, new_v) pair.  Internally the work is sharded across 8 NeuronCores
along the num_heads axis (32 heads -> 4 per core); each (batch, head) slab is
fully independent, so per core the kernel is just two big shifted DRAM->DRAM
DMA copies (bulk: out[:, :8176, :] = cache[:, 16:, :]) plus two tiny tail
copies from k_new / v_new.

Engine engagement: the HWDGE sprays one InstDMACopy's descriptors over SDMA
engine slots by the OUTER AP dimension.  A single [8, 32, 32704] AP (one DMA
for all 8 slabs) lands on only 8 of the 16 engines; issuing one FLAT 1-D DMA
per slab makes bass's single-dim balancer split it [32, 32704] (outer 32,
64 KiB descriptors), which round-robins over all 16 engine slots.
"""

import numpy as np

N_CORES = 8
B, H, S, D = 2, 32, 8192, 128
S_NEW = 16
KEEP = S - S_NEW  # 8176
HPC = H // N_CORES  # heads per core
BLK = B * HPC  # independent (batch, head) slabs per core

SLAB = S * D  # elements per slab (1048576)
BULK = KEEP * D  # bulk elements per slab (1046528)
TAIL = S_NEW * D  # tail elements per slab (2048)
NTOT = BLK * SLAB

_NC_CACHE = {}


def _build_nc():
    """Build the single-core Bass program (same program on all 8 cores)."""
    import concourse.bass as bass
    import concourse.mybir as mybir

    nc = bass.Bass()
    dt = mybir.dt.bfloat16
    # Flat 1-D tensors so each per-slab bulk copy presents a single-dim AP,
    # which balance_dma_aps splits [outer=32, 32704] -> all 16 SDMA engines.
    ck = nc.dram_tensor("cache_k", [NTOT], dt, kind="ExternalInput")
    cv = nc.dram_tensor("cache_v", [NTOT], dt, kind="ExternalInput")
    kn = nc.dram_tensor("k_new", [BLK * TAIL], dt, kind="ExternalInput")
    vn = nc.dram_tensor("v_new", [BLK * TAIL], dt, kind="ExternalInput")
    ok = nc.dram_tensor("out_k", [NTOT], dt, kind="ExternalOutput")
    ov = nc.dram_tensor("out_v", [NTOT], dt, kind="ExternalOutput")

    ok3 = ok.reshape([BLK, S, D])
    ov3 = ov.reshape([BLK, S, D])
    kn3 = kn.reshape([BLK, S_NEW, D])
    vn3 = vn.reshape([BLK, S_NEW, D])

    with nc.Block() as block, nc.semaphore("dma_sem") as dma_sem:

        @block.sync
        def _(sync):
            for i in range(BLK):
                sync.dma_start(
                    out=ok[i * SLAB : i * SLAB + BULK],
                    in_=ck[i * SLAB + TAIL : (i + 1) * SLAB],
                ).then_inc(dma_sem, 16)
            # Tail: the 16 new rows per slab (32 KiB total).
            sync.dma_start(out=ok3[:, KEEP:S, :], in_=kn3[:, :, :]).then_inc(
                dma_sem, 16
            )
            sync.wait_ge(dma_sem, 16 * (BLK + 1) * 2)

        @block.scalar
        def _(scalar):
            for i in range(BLK):
                scalar.dma_start(
                    out=ov[i * SLAB : i * SLAB + BULK],
                    in_=cv[i * SLAB + TAIL : (i + 1) * SLAB],
                ).then_inc(dma_sem, 16)
            scalar.dma_start(out=ov3[:, KEEP:S, :], in_=vn3[:, :, :]).then_inc(
                dma_sem, 16
            )

    return nc


def _get_nc():
    if "nc" not in _NC_CACHE:
        _NC_CACHE["nc"] = _build_nc()
    return _NC_CACHE["nc"]


def _shard(arr, c, n_rows):
    """Head-shard for core c, flattened to 1-D, contiguous."""
    sl = arr[:, c * HPC : (c + 1) * HPC]
    return np.ascontiguousarray(sl).reshape(-1)


def _run_spmd(cache_k, cache_v, k_new, v_new, trace=False, trace_kwargs=None):
    from concourse.bass_utils import run_bass_kernel_spmd

    nc = _get_nc()
    in_maps = [
        {
            "cache_k": _shard(cache_k, c, S),
            "cache_v": _shard(cache_v, c, S),
            "k_new": _shard(k_new, c, S_NEW),
            "v_new": _shard(v_new, c, S_NEW),
        }
        for c in range(N_CORES)
    ]
    kw = {}
    if trace:
        kw["trace"] = True
        if trace_kwargs:
            kw.update(trace_kwargs)
    return run_bass_kernel_spmd(nc, in_maps, core_ids=list(range(N_CORES)), **kw)


def _gather(results):
    out_k = np.concatenate(
        [results[c]["out_k"].reshape(B, HPC, S, D) for c in range(N_CORES)], axis=1
    )
    out_v = np.concatenate(
        [results[c]["out_v"].reshape(B, HPC, S, D) for c in range(N_CORES)], axis=1
    )
    return out_k, out_v


def kernel(cache_k, cache_v, k_new, v_new):
    cache_k = np.asarray(cache_k)
    cache_v = np.asarray(cache_v)
    k_new = np.asarray(k_new)
    v_new = np.asarray(v_new)
    res = _run_spmd(cache_k, cache_v, k_new, v_new)
    return _gather(res.results)
